# revision 1
# baseline (speedup 1.0000x reference)
"""Trainium2 Bass kernel for nn_ReallocationMapEncoder.

The reference network is three NAC layers (y = x @ (tanh(W_hat)*sigmoid(M_hat)).T)
applied to a [nsteps, nsyms, nsyms, 3] grid of normalized (t, a, b) indices,
plus a gb broadcast on the trailing axis. NAC is linear in x, so the whole
network collapses to one effective matrix Weff = W3 @ W2 @ W1 of shape [2, 3]:

    y[t, a, b, c] = gb[c] + (t/2)*Weff[c,0] + (a/2048)*Weff[c,1] + (b/2048)*Weff[c,2]

The output [2, 2048, 2048, 2] f32 (67 MB) is a separable affine ramp; the kernel
is purely output-write-bandwidth bound (memory regime).

Device strategy (8 cores, data-parallel on the `a` axis, 256 rows each, so
each core writes 8.4 MB): one DVE iota builds J[p, j] = j once; every output
slice [128 a-rows, 2048 b-cols at c-stride] is then a single fused DVE
tensor_scalar

    out[p, b, c] = J[p, b] * (Weff[c,2]/nsyms) + bias[p, (t,blk,c)]

where bias (a tiny [128, 8] per-core input) folds the gb/t/a terms:
bias[p, t,blk,c] = gb[c] + (t/2)*Weff[c,0] + (a(p,blk)/2048)*Weff[c,1].

Sync-wait slot limits in walrus codegen (HWDGE DMA: 1, DVE/ACT: 2) shape the
structure: single compute engine, one out-DMA per [128, 4096] tile, and at
most 8 total DMAs so DMAHW queues are never reused.
"""

import numpy as np

NSTEPS = 2
NSYMS = 2048
NCORES = 8
A_PER_CORE = NSYMS // NCORES          # 256
BLKS = A_PER_CORE // 128              # 2 partition blocks per core
F = NSYMS * 2                         # 4096 free elements per a-row (b, c interleaved)

_CACHE = {}


def _build_bass(scales):
    import concourse.bass as bass
    import concourse.mybir as mybir
    from concourse.tile import TileContext

    f32 = mybir.dt.float32
    nc = bass.Bass(trn_type="TRN2")

    bias_in = nc.dram_tensor("bias_in", [128, NSTEPS * BLKS * 2], f32, kind="ExternalInput")
    out = nc.dram_tensor("out", [NSTEPS, BLKS, 128, F], f32, kind="ExternalOutput")

    with TileContext(nc) as tc:
        with (
            tc.tile_pool(name="const", bufs=1) as const,
            tc.tile_pool(name="outp", bufs=4) as outp,
        ):
            bias_sb = const.tile([128, NSTEPS * BLKS * 2], f32)
            nc.gpsimd.dma_start(bias_sb[:], bias_in[:])

            J = const.tile([128, NSYMS], f32)
            nc.gpsimd.iota(
                J[:], pattern=[[1, NSYMS]], base=0, channel_multiplier=0,
                allow_small_or_imprecise_dtypes=True,
            )

            # This walrus build fits exactly ONE semaphore wait per
            # instruction. Two tiny observer copies make DVE's vector clock
            # see the iota (Pool sem) and the bias DMA (DMAHW sem) one at a
            # time, so the real tensor_scalar ops below need no waits at
            # all, and with bufs=4 no output slot is ever reused.
            scratch = const.tile([1, 2], f32)
            nc.vector.tensor_copy(scratch[:, 0:1], J[0:1, 0:1])
            nc.vector.tensor_copy(scratch[:, 1:2], bias_sb[0:1, 0:1])

            for t in range(NSTEPS):
                for blk in range(BLKS):
                    ot = outp.tile([128, F], f32)
                    otv = ot[:].rearrange("p (b c) -> p b c", c=2)
                    for c in range(2):
                        idx = (t * BLKS + blk) * 2 + c
                        nc.vector.tensor_scalar(
                            otv[:, :, c],
                            J[:],
                            scales[c],
                            bias_sb[:, idx : idx + 1],
                            mybir.AluOpType.mult,
                            mybir.AluOpType.add,
                        )
                    nc.gpsimd.dma_start(out[t, blk], ot[:])

    _legalize_waits(nc, mybir)
    return nc


def _legalize_waits(nc, mybir):
    """This walrus build fits very few semaphore waits per instruction (one
    for most engine structs). Tile's auto-generated kernel-tail drain waits
    on every DMA lane + engine sem at once; split any multi-wait instruction
    into a chain of single-wait Drain carriers on the same engine."""
    for func in nc.m.functions:
        for block in func.blocks:
            insts = list(block.instructions)
            new_insts = []
            changed = False
            for inst in insts:
                si = inst.sync_info
                waits = list(si.on_wait) if si is not None and si.on_wait else []
                if len(waits) > 1:
                    for w in waits[:-1]:
                        d = mybir.InstDrain(
                            name=f"{inst.name}-waitsplit-{len(new_insts)}",
                            ins=[],
                            outs=[],
                            bass_is_fusable=False,
                        )
                        d.engine = inst.engine
                        d.sync_info = mybir.SyncInfo(on_wait=[w], on_update=[])
                        new_insts.append(d)
                    inst.sync_info = mybir.SyncInfo(
                        on_wait=[waits[-1]], on_update=list(si.on_update or [])
                    )
                    changed = True
                new_insts.append(inst)
            if changed:
                block.instructions = new_insts


def _host_consts(gb, w_hat1, m_hat1, w_hat2, m_hat2, w_hat3, m_hat3):
    def nacw(w, m):
        w = np.asarray(w, np.float64)
        m = np.asarray(m, np.float64)
        return np.tanh(w) * (1.0 / (1.0 + np.exp(-m)))

    weff = nacw(w_hat3, m_hat3) @ nacw(w_hat2, m_hat2) @ nacw(w_hat1, m_hat1)  # [2,3]
    gb = np.asarray(gb, np.float64)

    scales = [float(np.float32(weff[c, 2] / NSYMS)) for c in range(2)]

    # bias[core][p, (t,blk,c)] = gb[c] + (t/2)Weff[c,0] + (a/2048)Weff[c,1]
    biases = []
    for core in range(NCORES):
        bias = np.empty((128, NSTEPS, BLKS, 2), np.float64)
        for t in range(NSTEPS):
            for blk in range(BLKS):
                a = (core * A_PER_CORE + blk * 128 + np.arange(128)) / NSYMS
                for c in range(2):
                    bias[:, t, blk, c] = (
                        gb[c] + (t / NSTEPS) * weff[c, 0] + a * weff[c, 1]
                    )
        biases.append(np.ascontiguousarray(bias.reshape(128, -1), np.float32))
    return scales, biases


def kernel(market, gb, w_hat1, m_hat1, w_hat2, m_hat2, w_hat3, m_hat3):
    from concourse.bass_utils import run_bass_kernel_spmd

    scales, biases = _host_consts(gb, w_hat1, m_hat1, w_hat2, m_hat2, w_hat3, m_hat3)
    # the tensor_scalar immediates (scales) are baked into the traced program,
    # so the compiled module is keyed on them
    key = ("nc", tuple(scales))
    if key not in _CACHE:
        _CACHE[key] = _build_bass(scales)
    nc = _CACHE[key]
    _CACHE["last_nc"] = nc

    in_maps = [{"bias_in": biases[core]} for core in range(NCORES)]
    res = run_bass_kernel_spmd(nc, in_maps, core_ids=list(range(NCORES)))
    parts = [r["out"].reshape(NSTEPS, A_PER_CORE, NSYMS, 2) for r in res.results]
    return np.concatenate(parts, axis=1)



# revision 2
# speedup vs baseline: 1.7515x; 1.7515x over previous
"""Trainium2 Bass kernel for nn_ReallocationMapEncoder.

The reference network is three NAC layers (y = x @ (tanh(W_hat)*sigmoid(M_hat)).T)
applied to a [nsteps, nsyms, nsyms, 3] grid of normalized (t, a, b) indices,
plus a gb broadcast on the trailing axis. NAC is linear in x, so the whole
network collapses to one effective matrix Weff = W3 @ W2 @ W1 of shape [2, 3]:

    y[t, a, b, c] = gb[c] + (t/2)*Weff[c,0] + (a/2048)*Weff[c,1] + (b/2048)*Weff[c,2]

The output [2, 2048, 2048, 2] (67 MB as f32) is a separable affine ramp; the
kernel is purely output-write-bandwidth bound (memory regime).

Device strategy (8 cores, data-parallel on the `a` axis, 256 rows each):
  * emit bf16 on device (the 2e-2 rel-err budget dwarfs bf16's 2^-9 rounding,
    and bf16 keeps f32's exponent range so near-zero outputs stay accurate);
    host upcasts to f32. Halves HBM write traffic vs f32: 4.2 MB/core.
  * the b-index ramp J comes in as a precomputed f32 input (the gpsimd iota
    used before cost 4.3us on the slow Q7 and serialized with its DMAs).
  * tiles are produced by TWO engines in parallel -- DVE tensor_scalar and
    ACT activation(Identity, bias=per-partition AP, scale=imm) compute
    out[p, b, c] = J[b]*(Weff[c,2]/nsyms) + bias[p, (t,blk,c)]
  * all DMAs ride the two HWDGE rings: sync (SP) carries the input loads and
    the DVE-produced chunks (one sem wait each -- within the HWDGE 1-wait
    budget); ACT issues its own chunks' DMAs in program order (zero waits).
    No SWDGE: the Q7 descriptor-generation path cost up to 8.3us per tile.

Sync-wait slot limits in walrus codegen (HWDGE DMA: 1, DVE/ACT: 2) shape the
structure: per-engine observer copies serialize the two input-DMA sems into
each compute engine's vector clock one at a time, after which compute ops and
ACT's own DMAs need no waits at all.
"""

import numpy as np

NSTEPS = 2
NSYMS = 2048
NCORES = 8
A_PER_CORE = NSYMS // NCORES          # 256
BLKS = A_PER_CORE // 128              # 2 partition blocks per core
HALVES = 2                            # column-split of each [128, 4096] row block
FB = NSYMS * 2 // HALVES              # 2048 free elements per chunk (b,c interleaved)
F = NSYMS * 2

_CACHE = {}


def _build_bass(scales):
    import concourse.bass as bass
    import concourse.mybir as mybir
    from concourse.tile import TileContext

    f32 = mybir.dt.float32
    bf16 = mybir.dt.bfloat16
    nc = bass.Bass(trn_type="TRN2")

    bias_in = nc.dram_tensor("bias_in", [128, NSTEPS * BLKS * 2], f32, kind="ExternalInput")
    jtab_in = nc.dram_tensor("jtab_in", [128, NSYMS], f32, kind="ExternalInput")
    out = nc.dram_tensor("out", [NSTEPS, BLKS, 128, F], bf16, kind="ExternalOutput")

    with TileContext(nc) as tc:
        with (
            tc.tile_pool(name="const", bufs=1) as const,
            tc.tile_pool(name="outp", bufs=8) as outp,
        ):
            bias_sb = const.tile([128, NSTEPS * BLKS * 2], f32)
            jtab_sb = const.tile([128, NSYMS], f32)
            nc.sync.dma_start(jtab_sb[:], jtab_in[:])
            nc.sync.dma_start(bias_sb[:], bias_in[:])

            # Observer copies: fold each input-DMA completion sem into the
            # engine's vector clock with a single-wait instruction, so the
            # real compute ops below need no waits.
            vscr = const.tile([1, 2], f32)
            sscr = const.tile([1, 2], f32)
            nc.vector.tensor_copy(vscr[:, 0:1], jtab_sb[0:1, 0:1])
            nc.vector.tensor_copy(vscr[:, 1:2], bias_sb[0:1, 0:1])
            nc.scalar.copy(sscr[:, 0:1], jtab_sb[0:1, 0:1])
            nc.scalar.copy(sscr[:, 1:2], bias_sb[0:1, 0:1])

            k = 0
            for t in range(NSTEPS):
                for blk in range(BLKS):
                    for h in range(HALVES):
                        ot = outp.tile([128, FB], bf16)
                        otv = ot[:].rearrange("p (b c) -> p b c", c=2)
                        jsl = jtab_sb[:, h * (FB // 2) : (h + 1) * (FB // 2)]
                        use_dve = (k % 2 == 0)
                        for c in range(2):
                            idx = (t * BLKS + blk) * 2 + c
                            bap = bias_sb[:, idx : idx + 1]
                            if use_dve:
                                nc.vector.tensor_scalar(
                                    otv[:, :, c], jsl, scales[c], bap,
                                    mybir.AluOpType.mult, mybir.AluOpType.add,
                                )
                            else:
                                nc.scalar.activation(
                                    otv[:, :, c], jsl,
                                    mybir.ActivationFunctionType.Identity,
                                    bias=bap, scale=scales[c],
                                )
                        dst = out[t, blk][:, h * FB : (h + 1) * FB]
                        if use_dve:
                            nc.sync.dma_start(dst, ot[:])
                        else:
                            nc.scalar.dma_start(dst, ot[:])
                        k += 1

    _legalize_waits(nc, mybir)
    return nc


def _legalize_waits(nc, mybir):
    """This walrus build fits very few semaphore waits per instruction (one
    for most engine structs). Tile's auto-generated kernel-tail drain waits
    on every DMA lane + engine sem at once; split any multi-wait instruction
    into a chain of single-wait Drain carriers on the same engine."""
    for func in nc.m.functions:
        for block in func.blocks:
            insts = list(block.instructions)
            new_insts = []
            changed = False
            for inst in insts:
                si = inst.sync_info
                waits = list(si.on_wait) if si is not None and si.on_wait else []
                if len(waits) > 1:
                    for w in waits[:-1]:
                        d = mybir.InstDrain(
                            name=f"{inst.name}-waitsplit-{len(new_insts)}",
                            ins=[],
                            outs=[],
                            bass_is_fusable=False,
                        )
                        d.engine = inst.engine
                        d.sync_info = mybir.SyncInfo(on_wait=[w], on_update=[])
                        new_insts.append(d)
                    inst.sync_info = mybir.SyncInfo(
                        on_wait=[waits[-1]], on_update=list(si.on_update or [])
                    )
                    changed = True
                new_insts.append(inst)
            if changed:
                block.instructions = new_insts


def _host_consts(gb, w_hat1, m_hat1, w_hat2, m_hat2, w_hat3, m_hat3):
    def nacw(w, m):
        w = np.asarray(w, np.float64)
        m = np.asarray(m, np.float64)
        return np.tanh(w) * (1.0 / (1.0 + np.exp(-m)))

    weff = nacw(w_hat3, m_hat3) @ nacw(w_hat2, m_hat2) @ nacw(w_hat1, m_hat1)  # [2,3]
    gb = np.asarray(gb, np.float64)

    scales = [float(np.float32(weff[c, 2] / NSYMS)) for c in range(2)]

    # bias[core][p, (t,blk,c)] = gb[c] + (t/2)Weff[c,0] + (a/2048)Weff[c,1]
    biases = []
    for core in range(NCORES):
        bias = np.empty((128, NSTEPS, BLKS, 2), np.float64)
        for t in range(NSTEPS):
            for blk in range(BLKS):
                a = (core * A_PER_CORE + blk * 128 + np.arange(128)) / NSYMS
                for c in range(2):
                    bias[:, t, blk, c] = (
                        gb[c] + (t / NSTEPS) * weff[c, 0] + a * weff[c, 1]
                    )
        biases.append(np.ascontiguousarray(bias.reshape(128, -1), np.float32))

    jtab = np.ascontiguousarray(
        np.broadcast_to(np.arange(NSYMS, dtype=np.float32), (128, NSYMS))
    )
    return scales, biases, jtab


def kernel(market, gb, w_hat1, m_hat1, w_hat2, m_hat2, w_hat3, m_hat3):
    from concourse.bass_utils import run_bass_kernel_spmd

    scales, biases, jtab = _host_consts(
        gb, w_hat1, m_hat1, w_hat2, m_hat2, w_hat3, m_hat3
    )
    # the tensor_scalar immediates (scales) are baked into the traced program,
    # so the compiled module is keyed on them
    key = ("nc", tuple(scales))
    if key not in _CACHE:
        _CACHE[key] = _build_bass(scales)
    nc = _CACHE[key]
    _CACHE["last_nc"] = nc

    in_maps = [
        {"bias_in": biases[core], "jtab_in": jtab} for core in range(NCORES)
    ]
    res = run_bass_kernel_spmd(nc, in_maps, core_ids=list(range(NCORES)))
    parts = [
        np.asarray(r["out"]).reshape(NSTEPS, A_PER_CORE, NSYMS, 2).astype(np.float32)
        for r in res.results
    ]
    return np.concatenate(parts, axis=1)
